# revision 5
# baseline (speedup 1.0000x reference)
"""Trainium2 Bass kernel for nn_Net_52209622450626.

Temporal-logic network scan: 288 sequential steps over state (B=64, T=256, K=32),
data-parallel over batch across 8 NeuronCores (8 batches/core).

Per-core layout: partitions p = 32*bo + k (bo in 0..3 batch groups), free
col = 2*t + bi (bi in 0..1), 512 cols + 2 zero guard columns so the t-shift
(nxt) is a free-dim AP offset by 2. K-shifts and the dense right_w
contraction are folded into 128x128 block-diagonal stationaries; S and Z are
accumulated in PSUM by TensorE. State is kept scaled (A' = 100*A) so the
myrelu update is a single scalar_tensor_tensor: A'_new = 99*clip01(Z) + Z.
"""

import numpy as np

B, T, K, V = 64, 256, 32, 32
NCORES = 8
BLOC = B // NCORES  # 8
NSTEP = T + K       # 288
FD = 2 * T          # 512 free columns per core


def _softmax(x, axis):
    x = x.astype(np.float64)
    m = np.max(x, axis=axis, keepdims=True)
    e = np.exp(x - m)
    return e / np.sum(e, axis=axis, keepdims=True)


def _host_prep(w_right, w_op):
    """Build stationaries (lhsT layout [k, j]) and the per-partition bias."""
    sm_op = _softmax(w_op, 1)
    op_w = sm_op[:, :5].T
    atom_w = sm_op[:, 5:].T                       # (V, K)
    right_w = _softmax(w_right, 1)[:, :-1].T      # (K, K): R[j] = sum_k A[k] rw[k, j]
    w1, w2, w3, w4 = op_w[1], op_w[2], op_w[3], op_w[4]

    bias = w1 - w2 - w4
    ca = w2 - w1 + 0.01 * w4
    cn = 0.01 * w4
    cg = w3
    beta = w2 + w4
    ccp = 0.99 * w4

    Sh = np.zeros((K, K))
    for j in range(K - 1):
        Sh[j + 1, j] = 1.0
    Id = np.eye(K)
    M1 = Sh * ca[None, :] + right_w * beta[None, :]
    M2 = Id * cn[None, :] + Sh * cg[None, :]
    Dg = Id * ccp[None, :]

    def blk(m):
        return np.kron(np.eye(4), m).astype(np.float32)

    wstat = np.stack([
        blk(atom_w),        # 0: atom contraction (lhsT[v, j])
        blk(Sh / 100.0),    # 1: S left term (scaled state)
        blk(Id / 100.0),    # 2: S nxt term + C0 injection
        blk(M1 / 100.0),    # 3: Z terms on A
        blk(M2 / 100.0),    # 4: Z terms on N
        blk(Dg),            # 5: Z until term on Cc (true scale)
    ])
    cvec = np.tile(bias, 4).astype(np.float32).reshape(128, 1)
    return wstat, cvec


def _to_T(x_core):
    """(8, 256, 32) -> (128, 512): out[32*bo+v, 2*t+bi] = x[2*bo+bi, t, v]."""
    return np.ascontiguousarray(
        x_core.reshape(4, 2, T, V).transpose(0, 3, 2, 1).reshape(128, FD))


def _from_T(outT):
    """(128, 512) -> (8, 256, 32)."""
    return np.ascontiguousarray(
        outT.reshape(4, K, T, 2).transpose(0, 3, 2, 1).reshape(BLOC, T, K))


def build_bass():
    import concourse.bacc as bacc
    import concourse.mybir as mybir
    from concourse.tile import TileContext

    f32 = mybir.dt.float32
    Alu = mybir.AluOpType

    nc = bacc.Bacc("TRN2", target_bir_lowering=False, debug=False)
    x_d = nc.dram_tensor("xT", [128, FD], f32, kind="ExternalInput")
    w_d = nc.dram_tensor("wstat", [6, 128, 128], f32, kind="ExternalInput")
    c_d = nc.dram_tensor("cvec", [128, 1], f32, kind="ExternalInput")
    y_d = nc.dram_tensor("outT", [128, FD], f32, kind="ExternalOutput")

    with TileContext(nc) as tc:
        with (
            tc.tile_pool(name="wp", bufs=1) as wp,
            tc.tile_pool(name="tmp", bufs=3) as tp,
            tc.tile_pool(name="psS", bufs=2, space="PSUM") as pS,
            tc.tile_pool(name="psZ", bufs=2, space="PSUM") as pZ,
            tc.tile_pool(name="psA", bufs=1, space="PSUM") as pA,
        ):
            wt = wp.tile([128, 6 * 128], f32, tag="w")
            for i in range(6):
                nc.sync.dma_start(wt[:, i * 128:(i + 1) * 128], w_d[i])
            cv = wp.tile([128, 1], f32, tag="cv")
            nc.sync.dma_start(cv[:], c_d[:])
            xt = wp.tile([128, FD], f32, tag="x")
            nc.sync.dma_start(xt[:], x_d[:])

            AtomW = wt[:, 0:128]
            Shd = wt[:, 128:256]
            Idm = wt[:, 256:384]
            M1 = wt[:, 384:512]
            M2 = wt[:, 512:640]
            Dg = wt[:, 640:768]

            atom_ps = pA.tile([128, FD], f32, tag="atom")
            nc.tensor.matmul(atom_ps[:], AtomW, xt[:], start=True, stop=True)
            # C0 = 100 * (atom + bias); injected each step through Idm (=I/100)
            C0 = wp.tile([128, FD], f32, tag="c0")
            nc.vector.tensor_scalar(C0[:], atom_ps[:], cv[:], 100.0,
                                    Alu.add, Alu.mult)

            A0 = wp.tile([128, FD + 2], f32, tag="A0")
            A1 = wp.tile([128, FD + 2], f32, tag="A1")
            nc.gpsimd.memset(A0[:], 0.0)
            nc.gpsimd.memset(A1[:], 0.0)
            states = [A0, A1]

            for i in range(NSTEP):
                A = states[i % 2]
                An = states[(i + 1) % 2]
                Sps = pS.tile([128, FD], f32, tag="S")
                Zps = pZ.tile([128, FD], f32, tag="Z")
                # S = left + nxt  (true scale)
                nc.tensor.matmul(Sps[:], Shd, A[:, 0:FD], start=True, stop=False)
                nc.tensor.matmul(Sps[:], Idm, A[:, 2:FD + 2], start=False, stop=True)
                # Z = C0'' + ca*L + beta*R + cn*N + cg*XL  (+ ccp*Cc below)
                nc.tensor.matmul(Zps[:], Idm, C0[:], start=True, stop=False)
                nc.tensor.matmul(Zps[:], M1, A[:, 0:FD], start=False, stop=False)
                nc.tensor.matmul(Zps[:], M2, A[:, 2:FD + 2], start=False, stop=False)
                # Cc = clip(S, 1, 2) = clip01(S-1) + 1 (the +1 is folded into bias)
                Cc = tp.tile([128, FD], f32, tag="Cc")
                nc.vector.tensor_scalar(Cc[:], Sps[:], 2.0, 1.0, Alu.min, Alu.max)
                nc.tensor.matmul(Zps[:], Dg, Cc[:], start=False, stop=True)
                # A'_new = 99*clip01(Z) + Z  (= 100*myrelu(Z))
                C1 = tp.tile([128, FD], f32, tag="C1")
                nc.vector.tensor_scalar(C1[:], Zps[:], 1.0, 0.0, Alu.min, Alu.max)
                nc.vector.scalar_tensor_tensor(An[:, 0:FD], C1[:], 99.0, Zps[:],
                                               Alu.mult, Alu.add)

            Afin = states[NSTEP % 2]
            yt = tp.tile([128, FD], f32, tag="y")
            sgb = wp.tile([128, 1], f32, tag="sgb")
            nc.gpsimd.memset(sgb[:], -2.5)
            nc.scalar.activation(yt[:], Afin[:, 0:FD],
                                 mybir.ActivationFunctionType.Sigmoid,
                                 bias=sgb[:], scale=0.05)
            nc.sync.dma_start(y_d[:], yt[:])

    nc.compile()
    return nc


def make_in_maps(x, w_right, w_op):
    wstat, cvec = _host_prep(np.asarray(w_right), np.asarray(w_op))
    x = np.asarray(x, dtype=np.float32)
    return [
        {"xT": _to_T(x[c * BLOC:(c + 1) * BLOC]), "wstat": wstat, "cvec": cvec}
        for c in range(NCORES)
    ]


def gather_out(results):
    return np.concatenate([_from_T(results[c]["outT"]) for c in range(NCORES)],
                          axis=0)


def kernel(x, w_right, w_op):
    from concourse.bass_utils import run_bass_kernel_spmd

    nc = build_bass()
    in_maps = make_in_maps(x, w_right, w_op)
    res = run_bass_kernel_spmd(nc, in_maps, core_ids=list(range(NCORES)))
    out = gather_out(res.results)
    return out.astype(np.float32)


# revision 10
# speedup vs baseline: 1.8381x; 1.8381x over previous
"""Trainium2 Bass kernel for nn_Net_52209622450626.

Temporal-logic network scan: 288 sequential steps over state (B=64, T=256, K=32),
data-parallel over batch across 8 NeuronCores (8 batches/core).

Per-core layout: partitions p = 32*bo + k (bo in 0..3 batch groups), free
col = 2*t + bi (bi in 0..1), 512 cols + 2 zero guard columns so the t-shift
(nxt) is a free-dim AP offset by 2. K-shifts and the dense right_w
contraction are folded into 128x128 block-diagonal stationaries; S and Z are
accumulated in PSUM by TensorE. State is kept scaled (A' = 100*A) so the
myrelu update is a single scalar_tensor_tensor: A'_new = 99*clip01(Z) + Z.
"""

import numpy as np

B, T, K, V = 64, 256, 32, 32
NCORES = 8
BLOC = B // NCORES  # 8
NSTEP = T + K       # 288
FD = 2 * T          # 512 free columns per core


def _softmax(x, axis):
    x = x.astype(np.float64)
    m = np.max(x, axis=axis, keepdims=True)
    e = np.exp(x - m)
    return e / np.sum(e, axis=axis, keepdims=True)


def _host_prep(w_right, w_op):
    """Build stationaries (lhsT layout [k, j]) and the per-partition bias."""
    sm_op = _softmax(w_op, 1)
    op_w = sm_op[:, :5].T
    atom_w = sm_op[:, 5:].T                       # (V, K)
    right_w = _softmax(w_right, 1)[:, :-1].T      # (K, K): R[j] = sum_k A[k] rw[k, j]
    w1, w2, w3, w4 = op_w[1], op_w[2], op_w[3], op_w[4]

    bias = w1 - w2 - w4
    ca = w2 - w1 + 0.01 * w4
    cn = 0.01 * w4
    cg = w3
    beta = w2 + w4
    ccp = 0.99 * w4

    Sh = np.zeros((K, K))
    for j in range(K - 1):
        Sh[j + 1, j] = 1.0
    Id = np.eye(K)
    M1 = Sh * ca[None, :] + right_w * beta[None, :]
    M2 = Id * cn[None, :] + Sh * cg[None, :]
    Dg = Id * ccp[None, :]

    def blk(m):
        return np.kron(np.eye(4), m).astype(np.float32)

    wstat = np.stack([
        blk(atom_w),        # 0: atom contraction (lhsT[v, j])
        blk(Sh / 100.0),    # 1: S left term (scaled state)
        blk(Id / 100.0),    # 2: S nxt term + C0 injection
        blk(M1 / 100.0),    # 3: Z terms on A
        blk(M2 / 100.0),    # 4: Z terms on N
        blk(Dg),            # 5: Z until term on Cc (true scale)
    ])
    cvec = np.tile(bias, 4).astype(np.float32).reshape(128, 1)
    return wstat, cvec


def _to_T(x_core):
    """(8, 256, 32) -> (128, 512): out[32*bo+v, 2*t+bi] = x[2*bo+bi, t, v]."""
    return np.ascontiguousarray(
        x_core.reshape(4, 2, T, V).transpose(0, 3, 2, 1).reshape(128, FD))


def _from_T(outT):
    """(128, 512) -> (8, 256, 32)."""
    return np.ascontiguousarray(
        outT.reshape(4, K, T, 2).transpose(0, 3, 2, 1).reshape(BLOC, T, K))


def build_bass():
    import concourse.bacc as bacc
    import concourse.mybir as mybir
    from concourse.tile import TileContext

    f32 = mybir.dt.float32
    Alu = mybir.AluOpType

    nc = bacc.Bacc("TRN2", target_bir_lowering=False, debug=False)
    f32r = mybir.dt.float32r
    x_d = nc.dram_tensor("xT", [128, FD], f32r, kind="ExternalInput")
    w_d = nc.dram_tensor("wstat", [6, 128, 128], f32r, kind="ExternalInput")
    c_d = nc.dram_tensor("cvec", [128, 1], f32, kind="ExternalInput")
    y_d = nc.dram_tensor("outT", [128, FD], f32, kind="ExternalOutput")

    with TileContext(nc) as tc:
        with (
            tc.tile_pool(name="wp", bufs=1) as wp,
            tc.tile_pool(name="tmp", bufs=3) as tp,
            tc.tile_pool(name="psS", bufs=2, space="PSUM") as pS,
            tc.tile_pool(name="psZ", bufs=2, space="PSUM") as pZ,
            tc.tile_pool(name="psA", bufs=1, space="PSUM") as pA,
        ):
            wt = wp.tile([128, 6 * 128], f32r, tag="w")
            for i in range(6):
                nc.sync.dma_start(wt[:, i * 128:(i + 1) * 128], w_d[i])
            cv = wp.tile([128, 1], f32, tag="cv")
            nc.sync.dma_start(cv[:], c_d[:])
            xt = wp.tile([128, FD], f32r, tag="x")
            nc.sync.dma_start(xt[:], x_d[:])

            # float32r: same fp32 bits, but PE streams at full rate (1 cyc/row
            # at moving dim >= 256) instead of fp32's quarter rate.
            wtr = wt
            AtomW = wtr[:, 0:128]
            Shd = wtr[:, 128:256]
            Idm = wtr[:, 256:384]
            M1 = wtr[:, 384:512]
            M2 = wtr[:, 512:640]
            Dg = wtr[:, 640:768]

            atom_ps = pA.tile([128, FD], f32, tag="atom")
            nc.tensor.matmul(atom_ps[:], AtomW, xt[:], start=True, stop=True)
            # C0 = 100 * (atom + bias); injected each step through Idm (=I/100)
            C0 = wp.tile([128, FD], f32r, tag="c0")
            nc.vector.tensor_scalar(C0[:], atom_ps[:], cv[:], 100.0,
                                    Alu.add, Alu.mult)

            A0 = wp.tile([128, FD + 2], f32r, tag="A0")
            A1 = wp.tile([128, FD + 2], f32r, tag="A1")
            # memset can't target f32r; zero-init via (x * 0) which is a
            # legal fp32r-rounded producer
            for St in (A0, A1):
                nc.vector.tensor_scalar(St[:, 0:FD], xt[:], 0.0, None, Alu.mult)
                nc.vector.tensor_scalar(St[:, FD:FD + 2], xt[:, 0:2], 0.0, None,
                                        Alu.mult)
            states = [A0, A1]

            for i in range(NSTEP):
                A = states[i % 2]
                An = states[(i + 1) % 2]
                Sps = pS.tile([128, FD], f32, tag="S")
                Zps = pZ.tile([128, FD], f32, tag="Z")
                # S = left + nxt  (true scale)
                nc.tensor.matmul(Sps[:], Shd, A[:, 0:FD], start=True, stop=False)
                nc.tensor.matmul(Sps[:], Idm, A[:, 2:FD + 2], start=False, stop=True)
                # Z = C0'' + ca*L + beta*R + cn*N + cg*XL  (+ ccp*Cc below)
                nc.tensor.matmul(Zps[:], Idm, C0[:], start=True, stop=False)
                nc.tensor.matmul(Zps[:], M1, A[:, 0:FD], start=False, stop=False)
                nc.tensor.matmul(Zps[:], M2, A[:, 2:FD + 2], start=False, stop=False)
                # Cc = clip(S, 1, 2) = clip01(S-1) + 1 (the +1 is folded into bias)
                Cc = tp.tile([128, FD], f32r, tag="Cc")
                nc.vector.tensor_scalar(Cc[:], Sps[:], 2.0, 1.0, Alu.min, Alu.max)
                nc.tensor.matmul(Zps[:], Dg, Cc[:], start=False, stop=True)
                # A'_new = 99*clip01(Z) + Z  (= 100*myrelu(Z))
                C1 = tp.tile([128, FD], f32, tag="C1")
                nc.vector.tensor_scalar(C1[:], Zps[:], 1.0, 0.0, Alu.min, Alu.max)
                nc.vector.scalar_tensor_tensor(An[:, 0:FD], C1[:], 99.0, Zps[:],
                                               Alu.mult, Alu.add)

            Afin = states[NSTEP % 2]
            yt = tp.tile([128, FD], f32, tag="y")
            sgb = wp.tile([128, 1], f32, tag="sgb")
            nc.gpsimd.memset(sgb[:], -2.5)
            nc.scalar.activation(yt[:], Afin[:, 0:FD].bitcast(f32),
                                 mybir.ActivationFunctionType.Sigmoid,
                                 bias=sgb[:], scale=0.05)
            nc.sync.dma_start(y_d[:], yt[:])

    nc.compile()
    return nc


def make_in_maps(x, w_right, w_op):
    wstat, cvec = _host_prep(np.asarray(w_right), np.asarray(w_op))
    x = np.asarray(x, dtype=np.float32)
    return [
        {"xT": _to_T(x[c * BLOC:(c + 1) * BLOC]), "wstat": wstat, "cvec": cvec}
        for c in range(NCORES)
    ]


def gather_out(results):
    return np.concatenate([_from_T(results[c]["outT"]) for c in range(NCORES)],
                          axis=0)


def kernel(x, w_right, w_op):
    from concourse.bass_utils import run_bass_kernel_spmd

    nc = build_bass()
    in_maps = make_in_maps(x, w_right, w_op)
    res = run_bass_kernel_spmd(nc, in_maps, core_ids=list(range(NCORES)))
    out = gather_out(res.results)
    return out.astype(np.float32)


# revision 11
# speedup vs baseline: 2.6413x; 1.4369x over previous
"""Trainium2 Bass kernel for nn_Net_52209622450626.

Temporal-logic network scan: 288 sequential steps over state (B=64, T=256, K=32),
data-parallel over batch across 8 NeuronCores (8 batches/core).

Per-core layout: partitions p = 32*bo + k (bo in 0..3 batch groups), free
col = 2*t + bi (bi in 0..1), 512 cols + 2 zero guard columns so the t-shift
(nxt) is a free AP offset by 2. K-shifts and the dense right_w contraction
are folded into 128x128 block-diagonal float32r stationaries; S (=left+nxt)
and Z (linear part) accumulate in PSUM via TensorE. State is kept scaled
(A' = 100*A). The whole nonlinear tail -- until-clip, its diagonal
accumulation, and myrelu -- is one fused custom DVE op:

    Y   = Z + ccp * clip01(S - 1)        (S-1 rides the ScalarE evac bias)
    A'  = Y + 99 * clip01(Y)             (= 100 * myrelu(Y))
"""

import numpy as np

B, T, K, V = 64, 256, 32, 32
NCORES = 8
BLOC = B // NCORES  # 8
NSTEP = T + K       # 288
FD = 2 * T          # 512 free columns per core

_OP_NAME = "UNTIL_MYRELU_ANT"


def _register_custom_op():
    """Register the fused until+myrelu DVE op in the concourse registries.

    out = Y + clip01(Y)*imm2,  Y = in0 + clip01(in1)*s0
    (in0 = Z partial in PSUM, in1 = S-1 in SBUF, s0 = per-partition ccp,
    imm2 = 99.)
    """
    import concourse.dve_ops as dom
    from concourse.dve_spec import Spec, Src0, Src1, C0, C2, Zero, One, \
        maxx, minn, lower
    from concourse.dve_uop import DveOpSpec

    if _OP_NAME in dom._SUB_OPCODE_FOR_NAME:
        return next(o for o in dom.OPS if o.name == _OP_NAME)

    Y = Src0 + minn(maxx(Src1, Zero), One) * C0
    body = Y + minn(maxx(Y, Zero), One) * C2

    def ref(in0, in1, s0, s1, imm2):
        y = (in0.astype(np.float32)
             + np.clip(in1.astype(np.float32), 0.0, 1.0) * s0).astype(np.float32)
        return (y + np.clip(y, 0.0, 1.0) * imm2).astype(np.float32)

    spec = Spec(body=body, reference=ref)
    row = max(dom._SUB_OPCODE_FOR_NAME.values()) + 1
    assert row < 0x20
    dom._SUB_OPCODE_FOR_NAME[_OP_NAME] = row
    sha = DveOpSpec(name=_OP_NAME, opcode=row, uops=lower(spec, ver="v3"),
                    rd1_en=True).sha("v3")
    op = dom.DveOp(_OP_NAME, spec, subdim=False, uops_sha={"v3": sha})
    dom.OPS.append(op)
    dom.CUSTOM_DVE_SPECS[_OP_NAME] = spec
    return op


def _softmax(x, axis):
    x = x.astype(np.float64)
    m = np.max(x, axis=axis, keepdims=True)
    e = np.exp(x - m)
    return e / np.sum(e, axis=axis, keepdims=True)


def _host_prep(w_right, w_op):
    """Stationaries (lhsT layout [k, j]) and per-partition const columns."""
    sm_op = _softmax(w_op, 1)
    op_w = sm_op[:, :5].T
    atom_w = sm_op[:, 5:].T                       # (V, K)
    right_w = _softmax(w_right, 1)[:, :-1].T      # (K, K)
    w1, w2, w3, w4 = op_w[1], op_w[2], op_w[3], op_w[4]

    bias = w1 - w2 - 0.01 * w4    # includes +ccp fold from clip12 -> clip01
    ca = w2 - w1 + 0.01 * w4
    cn = 0.01 * w4
    cg = w3
    beta = w2 + w4
    ccp = 0.99 * w4

    Sh = np.zeros((K, K))
    for j in range(K - 1):
        Sh[j + 1, j] = 1.0
    Id = np.eye(K)
    M1 = Sh * ca[None, :] + right_w * beta[None, :]
    M2 = Id * cn[None, :] + Sh * cg[None, :]

    def blk(m):
        return np.kron(np.eye(4), m).astype(np.float32)

    wstat = np.stack([
        blk(atom_w),        # 0: atom contraction
        blk(Sh / 100.0),    # 1: S left term (state is 100x)
        blk(Id / 100.0),    # 2: S nxt term + C0 injection
        blk(M1 / 100.0),    # 3: Z terms on A
        blk(M2 / 100.0),    # 4: Z terms on N
    ])
    cvec = np.stack([
        np.tile(bias, 4),                 # col 0: C0 bias
        np.tile(ccp, 4),                  # col 1: ccp (custom-op s0)
        np.full(128, -1.0),               # col 2: ScalarE evac bias (S - 1)
    ], axis=1).astype(np.float32)         # (128, 3)
    return wstat, cvec


def _to_T(x_core):
    """(8, 256, 32) -> (128, 512): out[32*bo+v, 2*t+bi] = x[2*bo+bi, t, v]."""
    return np.ascontiguousarray(
        x_core.reshape(4, 2, T, V).transpose(0, 3, 2, 1).reshape(128, FD))


def _from_T(outT):
    """(128, 512) -> (8, 256, 32)."""
    return np.ascontiguousarray(
        outT.reshape(4, K, T, 2).transpose(0, 3, 2, 1).reshape(BLOC, T, K))


def build_bass():
    import concourse.bacc as bacc
    import concourse.mybir as mybir
    from concourse.tile import TileContext

    f32 = mybir.dt.float32
    Alu = mybir.AluOpType
    myop = _register_custom_op()

    nc = bacc.Bacc("TRN2", target_bir_lowering=False, debug=False)
    f32r = mybir.dt.float32r
    x_d = nc.dram_tensor("xT", [128, FD], f32r, kind="ExternalInput")
    w_d = nc.dram_tensor("wstat", [5, 128, 128], f32r, kind="ExternalInput")
    c_d = nc.dram_tensor("cvec", [128, 3], f32, kind="ExternalInput")
    y_d = nc.dram_tensor("outT", [128, FD], f32, kind="ExternalOutput")

    with TileContext(nc) as tc:
        with (
            tc.tile_pool(name="wp", bufs=1) as wp,
            tc.tile_pool(name="tmp", bufs=3) as tp,
            tc.tile_pool(name="psS", bufs=2, space="PSUM") as pS,
            tc.tile_pool(name="psZ", bufs=2, space="PSUM") as pZ,
            tc.tile_pool(name="psA", bufs=1, space="PSUM") as pA,
        ):
            wt = wp.tile([128, 5 * 128], f32r, tag="w")
            for i in range(5):
                nc.sync.dma_start(wt[:, i * 128:(i + 1) * 128], w_d[i])
            cv = wp.tile([128, 3], f32, tag="cv")
            nc.sync.dma_start(cv[:], c_d[:])
            xt = wp.tile([128, FD], f32r, tag="x")
            nc.sync.dma_start(xt[:], x_d[:])

            AtomW = wt[:, 0:128]
            Shd = wt[:, 128:256]
            Idm = wt[:, 256:384]
            M1 = wt[:, 384:512]
            M2 = wt[:, 512:640]

            atom_ps = pA.tile([128, FD], f32, tag="atom")
            nc.tensor.matmul(atom_ps[:], AtomW, xt[:], start=True, stop=True)
            # C0 = 100 * (atom + bias); injected each step through Idm (=I/100)
            C0 = wp.tile([128, FD], f32r, tag="c0")
            nc.vector.tensor_scalar(C0[:], atom_ps[:], cv[:, 0:1], 100.0,
                                    Alu.add, Alu.mult)

            A0 = wp.tile([128, FD + 2], f32r, tag="A0")
            A1 = wp.tile([128, FD + 2], f32r, tag="A1")
            for St in (A0, A1):
                nc.vector.tensor_scalar(St[:, 0:FD], xt[:], 0.0, None, Alu.mult)
                nc.vector.tensor_scalar(St[:, FD:FD + 2], xt[:, 0:2], 0.0, None,
                                        Alu.mult)
            states = [A0, A1]

            for i in range(NSTEP):
                A = states[i % 2]
                An = states[(i + 1) % 2]
                Sps = pS.tile([128, FD], f32, tag="S")
                Zps = pZ.tile([128, FD], f32, tag="Z")
                # S = left + nxt  (true scale)
                nc.tensor.matmul(Sps[:], Shd, A[:, 0:FD], start=True, stop=False)
                nc.tensor.matmul(Sps[:], Idm, A[:, 2:FD + 2], start=False, stop=True)
                # Z = C0'' + ca*L + beta*R + cn*N + cg*XL
                nc.tensor.matmul(Zps[:], Idm, C0[:], start=True, stop=False)
                nc.tensor.matmul(Zps[:], M1, A[:, 0:FD], start=False, stop=False)
                nc.tensor.matmul(Zps[:], M2, A[:, 2:FD + 2], start=False, stop=True)
                # ScalarE evacuates S with the -1 fold
                Sc = tp.tile([128, FD], f32, tag="Sc")
                nc.scalar.activation(Sc[:], Sps[:],
                                     mybir.ActivationFunctionType.Identity,
                                     bias=cv[:, 2:3], scale=1.0)
                # fused: A' = Y + 99*clip01(Y), Y = Z + ccp*clip01(S-1)
                nc.vector._custom_dve(myop, out=An[:, 0:FD], in0=Zps[:],
                                      in1=Sc[:], s0=cv[:, 1:2], s1=0.0,
                                      imm2=99.0)

            Afin = states[NSTEP % 2]
            yt = tp.tile([128, FD], f32, tag="y")
            sgb = wp.tile([128, 1], f32, tag="sgb")
            nc.gpsimd.memset(sgb[:], -2.5)
            nc.scalar.activation(yt[:], Afin[:, 0:FD].bitcast(f32),
                                 mybir.ActivationFunctionType.Sigmoid,
                                 bias=sgb[:], scale=0.05)
            nc.sync.dma_start(y_d[:], yt[:])

    nc.compile()
    return nc


def make_in_maps(x, w_right, w_op):
    wstat, cvec = _host_prep(np.asarray(w_right), np.asarray(w_op))
    x = np.asarray(x, dtype=np.float32)
    return [
        {"xT": _to_T(x[c * BLOC:(c + 1) * BLOC]), "wstat": wstat, "cvec": cvec}
        for c in range(NCORES)
    ]


def gather_out(results):
    return np.concatenate([_from_T(results[c]["outT"]) for c in range(NCORES)],
                          axis=0)


def kernel(x, w_right, w_op):
    from concourse.bass_utils import run_bass_kernel_spmd

    nc = build_bass()
    in_maps = make_in_maps(x, w_right, w_op)
    res = run_bass_kernel_spmd(nc, in_maps, core_ids=list(range(NCORES)))
    out = gather_out(res.results)
    return out.astype(np.float32)


# revision 13
# speedup vs baseline: 3.6398x; 1.3781x over previous
"""Trainium2 Bass kernel for nn_Net_52209622450626.

Temporal-logic network scan: 288 sequential steps over state (B=64, T=256, K=32),
data-parallel over batch across 8 NeuronCores (8 batches/core).

Per-core layout: partitions p = 32*bo + k (bo in 0..3 batch groups), free
col = 2*t + bi (bi in 0..1), 512 cols + 2 zero guard columns so the t-shift
(nxt) is a free AP offset by 2. K-shifts and the dense right_w contraction
are folded into 128x128 block-diagonal float32r stationaries; S (=left+nxt)
and Z (linear part) accumulate in PSUM via TensorE. State is kept scaled
(A' = 100*A). The whole nonlinear tail -- until-clip, its diagonal
accumulation, and myrelu -- is one fused custom DVE op:

    Y   = Z + ccp * clip01(S - 1)        (S-1 rides the ScalarE evac bias)
    A'  = Y + 99 * clip01(Y)             (= 100 * myrelu(Y))
"""

import numpy as np

B, T, K, V = 64, 256, 32, 32
NCORES = 8
BLOC = B // NCORES  # 8
NSTEP = T + K       # 288
FD = 2 * T          # 512 free columns per core

_OP_NAME = "UNTIL_MYRELU_ANT"


def _register_custom_op():
    """Register the fused until+myrelu DVE op in the concourse registries.

    out = Y + clip01(Y)*imm2,  Y = in0 + clip01(in1)*s0
    (in0 = Z partial in PSUM, in1 = S-1 in SBUF, s0 = per-partition ccp,
    imm2 = 99.)
    """
    import concourse.dve_ops as dom
    from concourse.dve_spec import Spec, Src0, Src1, C0, C2, Zero, One, \
        maxx, minn, lower
    from concourse.dve_uop import DveOpSpec

    if _OP_NAME in dom._SUB_OPCODE_FOR_NAME:
        return next(o for o in dom.OPS if o.name == _OP_NAME)

    Y = Src0 + minn(maxx(Src1, Zero), One) * C0
    body = Y + minn(maxx(Y, Zero), One) * C2

    def ref(in0, in1, s0, s1, imm2):
        y = (in0.astype(np.float32)
             + np.clip(in1.astype(np.float32), 0.0, 1.0) * s0).astype(np.float32)
        return (y + np.clip(y, 0.0, 1.0) * imm2).astype(np.float32)

    spec = Spec(body=body, reference=ref)
    row = max(dom._SUB_OPCODE_FOR_NAME.values()) + 1
    assert row < 0x20
    dom._SUB_OPCODE_FOR_NAME[_OP_NAME] = row
    sha = DveOpSpec(name=_OP_NAME, opcode=row, uops=lower(spec, ver="v3"),
                    rd1_en=True).sha("v3")
    op = dom.DveOp(_OP_NAME, spec, subdim=False, uops_sha={"v3": sha})
    dom.OPS.append(op)
    dom.CUSTOM_DVE_SPECS[_OP_NAME] = spec
    return op


def _softmax(x, axis):
    x = x.astype(np.float64)
    m = np.max(x, axis=axis, keepdims=True)
    e = np.exp(x - m)
    return e / np.sum(e, axis=axis, keepdims=True)


def _host_prep(w_right, w_op):
    """Stationaries (lhsT layout [k, j]) and per-partition const columns."""
    sm_op = _softmax(w_op, 1)
    op_w = sm_op[:, :5].T
    atom_w = sm_op[:, 5:].T                       # (V, K)
    right_w = _softmax(w_right, 1)[:, :-1].T      # (K, K)
    w1, w2, w3, w4 = op_w[1], op_w[2], op_w[3], op_w[4]

    bias = w1 - w2 - 0.01 * w4    # includes +ccp fold from clip12 -> clip01
    ca = w2 - w1 + 0.01 * w4
    cn = 0.01 * w4
    cg = w3
    beta = w2 + w4
    ccp = 0.99 * w4

    Sh = np.zeros((K, K))
    for j in range(K - 1):
        Sh[j + 1, j] = 1.0
    Id = np.eye(K)
    M1 = Sh * ca[None, :] + right_w * beta[None, :]
    M2 = Id * cn[None, :] + Sh * cg[None, :]

    def blk(m):
        return np.kron(np.eye(4), m).astype(np.float32)

    wstat = np.stack([
        blk(atom_w),        # 0: atom contraction
        blk(Sh / 100.0),    # 1: S left term (state is 100x)
        blk(Id / 100.0),    # 2: S nxt term + C0 injection
        blk(M1 / 100.0),    # 3: Z terms on A
        blk(M2 / 100.0),    # 4: Z terms on N
    ])
    cvec = np.stack([
        np.tile(bias, 4),                 # col 0: C0 bias
        np.tile(ccp, 4),                  # col 1: ccp (custom-op s0)
        np.full(128, -1.0),               # col 2: ScalarE evac bias (S - 1)
    ], axis=1).astype(np.float32)         # (128, 3)
    return wstat, cvec


def _to_T(x_core):
    """(8, 256, 32) -> (128, 512): out[32*bo+v, 2*t+bi] = x[2*bo+bi, t, v]."""
    return np.ascontiguousarray(
        x_core.reshape(4, 2, T, V).transpose(0, 3, 2, 1).reshape(128, FD))


def _from_T(outT):
    """(128, 512) -> (8, 256, 32)."""
    return np.ascontiguousarray(
        outT.reshape(4, K, T, 2).transpose(0, 3, 2, 1).reshape(BLOC, T, K))


def build_bass():
    import concourse.bacc as bacc
    import concourse.mybir as mybir
    from concourse.tile import TileContext

    f32 = mybir.dt.float32
    Alu = mybir.AluOpType
    myop = _register_custom_op()

    nc = bacc.Bacc("TRN2", target_bir_lowering=False, debug=False)
    f32r = mybir.dt.float32r
    x_d = nc.dram_tensor("xT", [128, FD], f32r, kind="ExternalInput")
    w_d = nc.dram_tensor("wstat", [5, 128, 128], f32r, kind="ExternalInput")
    c_d = nc.dram_tensor("cvec", [128, 3], f32, kind="ExternalInput")
    y_d = nc.dram_tensor("outT", [128, FD], f32, kind="ExternalOutput")

    with TileContext(nc) as tc:
        with (
            tc.tile_pool(name="wp", bufs=1) as wp,
            tc.tile_pool(name="tmp", bufs=4) as tp,
            tc.tile_pool(name="psS", bufs=4, space="PSUM") as pS,
            tc.tile_pool(name="psZ", bufs=4, space="PSUM") as pZ,
        ):
            wt = wp.tile([128, 5 * 128], f32r, tag="w")
            for i in range(5):
                nc.sync.dma_start(wt[:, i * 128:(i + 1) * 128], w_d[i])
            cv = wp.tile([128, 3], f32, tag="cv")
            nc.sync.dma_start(cv[:], c_d[:])
            xt = wp.tile([128, FD], f32r, tag="x")
            nc.sync.dma_start(xt[:], x_d[:])

            AtomW = wt[:, 0:128]
            Shd = wt[:, 128:256]
            Idm = wt[:, 256:384]
            M1 = wt[:, 384:512]
            M2 = wt[:, 512:640]

            atom_ps = pZ.tile([128, FD], f32, tag="Z")
            nc.tensor.matmul(atom_ps[:], AtomW, xt[:], start=True, stop=True)
            # C0 = 100 * (atom + bias); injected each step through Idm (=I/100)
            C0 = wp.tile([128, FD], f32r, tag="c0")
            nc.vector.tensor_scalar(C0[:], atom_ps[:], cv[:, 0:1], 100.0,
                                    Alu.add, Alu.mult)

            A0 = wp.tile([128, FD + 2], f32r, tag="A0")
            A1 = wp.tile([128, FD + 2], f32r, tag="A1")
            for St in (A0, A1):
                nc.vector.tensor_scalar(St[:, 0:FD], xt[:], 0.0, None, Alu.mult)
                nc.vector.tensor_scalar(St[:, FD:FD + 2], xt[:, 0:2], 0.0, None,
                                        Alu.mult)
            states = [A0, A1]

            HC = FD // 2   # 256-column chunks; chunk 1 (cols 256:512) first
            for i in range(NSTEP):
                A = states[i % 2]
                An = states[(i + 1) % 2]
                for c0 in (HC, 0):
                    c1 = c0 + HC
                    Sps = pS.tile([128, HC], f32, tag="S")
                    Zps = pZ.tile([128, HC], f32, tag="Z")
                    # S = left + nxt  (true scale)
                    nc.tensor.matmul(Sps[:], Shd, A[:, c0:c1],
                                     start=True, stop=False)
                    nc.tensor.matmul(Sps[:], Idm, A[:, c0 + 2:c1 + 2],
                                     start=False, stop=True)
                    # Z = C0'' + ca*L + beta*R + cn*N + cg*XL
                    nc.tensor.matmul(Zps[:], Idm, C0[:, c0:c1],
                                     start=True, stop=False)
                    nc.tensor.matmul(Zps[:], M1, A[:, c0:c1],
                                     start=False, stop=False)
                    nc.tensor.matmul(Zps[:], M2, A[:, c0 + 2:c1 + 2],
                                     start=False, stop=True)
                    # ScalarE evacuates S with the -1 fold
                    Sc = tp.tile([128, HC], f32, tag="Sc")
                    nc.scalar.activation(Sc[:], Sps[:],
                                         mybir.ActivationFunctionType.Identity,
                                         bias=cv[:, 2:3], scale=1.0)
                    # fused: A' = Y + 99*clip01(Y), Y = Z + ccp*clip01(S-1)
                    nc.vector._custom_dve(myop, out=An[:, c0:c1], in0=Zps[:],
                                          in1=Sc[:], s0=cv[:, 1:2], s1=0.0,
                                          imm2=99.0)

            Afin = states[NSTEP % 2]
            yt = tp.tile([128, FD], f32, tag="y")
            sgb = wp.tile([128, 1], f32, tag="sgb")
            nc.gpsimd.memset(sgb[:], -2.5)
            nc.scalar.activation(yt[:], Afin[:, 0:FD].bitcast(f32),
                                 mybir.ActivationFunctionType.Sigmoid,
                                 bias=sgb[:], scale=0.05)
            nc.sync.dma_start(y_d[:], yt[:])

    nc.compile()
    return nc


def make_in_maps(x, w_right, w_op):
    wstat, cvec = _host_prep(np.asarray(w_right), np.asarray(w_op))
    x = np.asarray(x, dtype=np.float32)
    return [
        {"xT": _to_T(x[c * BLOC:(c + 1) * BLOC]), "wstat": wstat, "cvec": cvec}
        for c in range(NCORES)
    ]


def gather_out(results):
    return np.concatenate([_from_T(results[c]["outT"]) for c in range(NCORES)],
                          axis=0)


def kernel(x, w_right, w_op):
    from concourse.bass_utils import run_bass_kernel_spmd

    nc = build_bass()
    in_maps = make_in_maps(x, w_right, w_op)
    res = run_bass_kernel_spmd(nc, in_maps, core_ids=list(range(NCORES)))
    out = gather_out(res.results)
    return out.astype(np.float32)
